# revision 33
# baseline (speedup 1.0000x reference)
"""Bicameral attention (8 local-causal + 8 global heads) on 8 TRN2 NeuronCores.

Sharding: core i handles batch b = i//2 and head-half j = i%2, i.e. 4 local
heads {4j..4j+3} and 4 global heads {8+4j..8+4j+3} — identical SPMD program,
per-core data prepared on the host.

Per-core device pipeline (all matmuls bf16, fp32 PSUM accumulation):
  1. qkT [1024, L]  = w_qk.T @ xT   (Q^T and K^T, head-dim on partitions;
     biases folded in with K=1 rank-1 matmuls; softmax scale folded into Q^T)
     v   [L, 520]   = xT.T @ w_v    (65-wide per head: 64 V columns + ones
     column so attn@V also produces softmax denominators)
  2. per (head-pair, 512-wide q-chunk):
     S^T tiles [k,q] via row-packed K=64 matmul pairs -> exp on ACT -> bf16
     E^T in SBUF -> causal mask multiply (local heads, diagonal band only)
     -> attn@V with M=65 (PSUM row 64 = denominators) -> reciprocal ->
     gpsimd partition_broadcast -> DVE normalizes E^T in place and the
     O^T slice into ocT -> one contiguous DMA of [128, nk*512] to DRAM.
     Fully-masked k-tiles are never computed or written (host leaves zeros).
  3. out_p [L, 1024] = ocT.T @ w2 — partial output projection; host sums the
     two cores of each batch and adds out_b.

Host reassembles attn blocks (transposing [k,q] -> [q,k]) and upcasts to f32.
"""

import math
import numpy as np
import ml_dtypes

import concourse.bass as bass
import concourse.bacc as bacc
import concourse.mybir as mybir
from concourse import tile
from concourse.bass_utils import run_bass_kernel_spmd
from concourse.vector_clock import ScopedClock

BF16 = ml_dtypes.bfloat16
F32 = np.float32

B, L, D = 4, 2048, 1024
H, HD = 16, 64
LH = H // 2              # 8 local heads / 8 global heads
CH = 8                   # heads per core (4 local + 4 global)
NQC = 4                  # q-chunks of 512
QC = 512
KT = 128                 # k-tile size
NKT = L // KT            # 16 k-tiles
DT = mybir.dt.bfloat16
DTF = mybir.dt.float32
AF = mybir.ActivationFunctionType
ALU = mybir.AluOpType


def _nk(ch, c):
    """Number of computed k-tiles for core-head ch, q-chunk c."""
    return 4 * (c + 1) if ch < 4 else NKT


# DRAM layout of the attn output: per (ch, c) a contiguous [128, nk*512]
# block (partition-major), concatenated.
_BLOCK_OFFS = {}
_TOTAL_ATTN = 0


def _init_offs():
    global _TOTAL_ATTN
    off = 0
    for ch in range(CH):
        for c in range(NQC):
            _BLOCK_OFFS[(ch, c)] = off
            off += 128 * _nk(ch, c) * QC
    _TOTAL_ATTN = off


_init_offs()


# ---------------------------------------------------------------------------
# Workaround: walrus CoreV3 codegen accepts at most one sync-wait on a Drain.
# Move the TileContext tail-drain waits onto individual wait_ge instructions.
# ---------------------------------------------------------------------------
def _patched_drain_and_barrier(self, tick_clock, wait_clock):
    nc = self.nc
    drain_inst = nc.sync.drain()
    wait_clock.add_sem_waits(
        drain_inst.ins, ScopedClock({None: tick_clock.global_clock})
    )
    waits = list(drain_inst.ins.sync_info.on_wait)
    if len(waits) > 1:
        drain_inst.ins.sync_info.on_wait.clear()
        num_to_handle = {h.num: h for h in self.sems.allocated().values()}
        for w in waits:
            handle = num_to_handle.get(w.id)
            assert handle is not None, f"unknown tail sem {w.ant_name}"
            assert w.wait_mode == "sem-ge-imm", w.wait_mode
            nc.sync.wait_ge(handle, w.wait_value)
    nc.all_engine_barrier()
    popped = nc._tile_sem_poison_stack.pop()
    assert popped is self._sem_poison
    nc.clear_and_free_semaphores(list(self.sems.allocated().values()))
    nc.all_engine_barrier()


tile.TileContext._drain_and_barrier = _patched_drain_and_barrier


# The kernel alternates Exp (softmax) and Ln (reciprocal via exp(-ln(d))).
# Left alone, insert_act_table_loads maps Exp to `exp_and_others` and Ln to a
# natural-log set, reloading the ACT spline tables (~1.3us each) dozens of
# times. `natural_log_exp_and_others` holds both; restrict Exp/Ln to it (set
# ids are positional, so contents are filtered rather than reordered).
from concourse.hw_specs import get_activation_tables as _orig_act_tables


def _patched_act_tables(arch):
    keep = "natural_log_exp_and_others"
    out = {}
    for name, fns in _orig_act_tables(arch).items():
        if name != keep:
            fns = {f for f in fns if f not in (AF.Exp, AF.Ln)}
        out[name] = set(fns)
    return out


bacc.get_activation_tables = _patched_act_tables


# ---------------------------------------------------------------------------
# Device program (identical on every core)
# ---------------------------------------------------------------------------
def _build_bass(has_bias):
    nc = bacc.Bacc("TRN2", target_bir_lowering=False, debug=False, num_devices=8)

    xT_d = nc.dram_tensor("xT", [D, L], DT, kind="ExternalInput").ap()
    wqk_d = nc.dram_tensor("w_qk", [D, 1024], DT, kind="ExternalInput").ap()
    wv_d = nc.dram_tensor("w_v", [D, 512], DT, kind="ExternalInput").ap()
    bqk_d = nc.dram_tensor("b_qk", [1, 1024], DT, kind="ExternalInput").ap()
    bv_d = nc.dram_tensor("b_v", [1, 512], DT, kind="ExternalInput").ap()
    scl_d = nc.dram_tensor("scl_q", [512, 1], DTF, kind="ExternalInput").ap()
    w2_d = nc.dram_tensor("w2", [512, 1024], DT, kind="ExternalInput").ap()
    masks_d = nc.dram_tensor("masks", [128, 2048], DT, kind="ExternalInput").ap()

    attn_d = nc.dram_tensor("attn_t", [_TOTAL_ATTN], DT, kind="ExternalOutput").ap()
    outp_d = nc.dram_tensor("out_p", [L, D], DTF, kind="ExternalOutput").ap()

    with tile.TileContext(nc) as tc:
        _emit(nc, tc, xT_d, wqk_d, wv_d, bqk_d, bv_d, scl_d, w2_d, masks_d,
              attn_d, outp_d, has_bias)
    nc.compile()
    return nc


def _emit(nc, tc, xT_d, wqk_d, wv_d, bqk_d, bv_d, scl_d, w2_d, masks_d,
          attn_d, outp_d, has_bias):
    from contextlib import ExitStack

    ctx = ExitStack()
    with ctx:
        persist = ctx.enter_context(tc.tile_pool(name="persist", bufs=1))

        # ---- persistent tiles ------------------------------------------------
        qk = [persist.tile([128, L], DT, tag=f"qk{t}", name=f"qk{t}") for t in range(8)]
        v_aug = [persist.tile([128, 520], DT, tag=f"v{t}", name=f"v{t}") for t in range(NKT)]
        oc = [persist.tile([128, L], DT, tag=f"oc{t}", name=f"oc{t}") for t in range(4)]
        w2 = [persist.tile([128, 1024], DT, tag=f"w2{t}", name=f"w2{t}") for t in range(4)]
        masks = persist.tile([128, 2048], DT, tag="masks", name="masks")
        scl = [persist.tile([128, 1], DTF, tag=f"scl{t}", name=f"scl{t}") for t in range(4)]
        ones = persist.tile([1, 512], DT, tag="ones", name="ones")
        bqk = persist.tile([1, 1024], DT, tag="bqk", name="bqk")
        bv = persist.tile([1, 512], DT, tag="bv", name="bv")

        nc.vector.memset(ones[:], 1.0)

        # ---- phase 1: qkv projection ----------------------------------------
        with tc.tile_pool(name="ph1", bufs=1) as ph1, \
             tc.tile_pool(name="p1ps", bufs=4, space="PSUM") as p1ps:
            xt = [ph1.tile([128, L], DT, tag=f"xt{d}", name=f"xt{d}") for d in range(8)]
            wqk = [ph1.tile([128, 1024], DT, tag=f"wqk{d}", name=f"wqk{d}") for d in range(8)]
            wv = [ph1.tile([128, 512], DT, tag=f"wv{d}", name=f"wv{d}") for d in range(8)]
            # first-needed tiles first: the very first matmul touches only
            # xt[0] + wqk[0]; bias/scale rows follow shortly after.
            nc.sync.dma_start(xt[0][:], xT_d[0:128, :])
            nc.sync.dma_start(wqk[0][:], wqk_d[0:128, :])
            if has_bias:
                nc.sync.dma_start(bqk[:], bqk_d[:])
            for t in range(4):
                nc.sync.dma_start(scl[t][:], scl_d[t * 128:(t + 1) * 128, :])
            for d in range(1, 8):
                nc.sync.dma_start(xt[d][:], xT_d[d * 128:(d + 1) * 128, :])
                nc.sync.dma_start(wqk[d][:], wqk_d[d * 128:(d + 1) * 128, :])
            for d in range(8):
                nc.sync.dma_start(wv[d][:], wv_d[d * 128:(d + 1) * 128, :])
            if has_bias:
                nc.sync.dma_start(bv[:], bv_d[:])
            nc.sync.dma_start(masks[:], masks_d[:])
            for t in range(4):
                nc.sync.dma_start(w2[t][:], w2_d[t * 128:(t + 1) * 128, :])

            # qkT[c, l] += w_qk[:, c].T @ xT ; optional bias via K=1 outer
            # product. Q/K c-tiles interleaved so head-pair 0's operands
            # finish first, with V L-tiles woven in so early attn@V chunks
            # (local heads need only the first few v tiles) aren't blocked
            # behind the whole projection.
            def emit_qk_ct(ct):
                csl = slice(ct * 128, (ct + 1) * 128)
                for lc in range(NQC):
                    lsl = slice(lc * QC, (lc + 1) * QC)
                    ps = p1ps.tile([128, QC], DTF, tag="ps", name="ps")
                    for d in range(8):
                        nc.tensor.matmul(ps[:], wqk[d][:, csl], xt[d][:, lsl],
                                         start=(d == 0), stop=not has_bias and d == 7)
                    if has_bias:
                        nc.tensor.matmul(ps[:], bqk[0:1, csl], ones[0:1, 0:QC],
                                         start=False, stop=True)
                    if ct < 4:  # Q^T rows: fold softmax scale
                        nc.vector.tensor_scalar(qk[ct][:, lsl], ps[:],
                                                scl[ct][:], None, ALU.mult)
                    else:
                        nc.vector.tensor_copy(qk[ct][:, lsl], ps[:])

            def emit_v_lt(lt):
                # v[l, 520] (65 per head: 64 cols + ones col)
                lsl = slice(lt * 128, (lt + 1) * 128)
                ps = p1ps.tile([128, 512], DTF, tag="psv", name="psv")
                for d in range(8):
                    nc.tensor.matmul(ps[:], xt[d][:, lsl], wv[d][:],
                                     start=(d == 0), stop=not has_bias and d == 7)
                if has_bias:
                    nc.tensor.matmul(ps[:], ones[0:1, 0:128], bv[0:1, :],
                                     start=False, stop=True)
                dst = v_aug[lt][:].rearrange("p (h c) -> p h c", c=65)
                nc.vector.tensor_copy(dst[:, :, 0:64],
                                      ps[:].rearrange("p (h c) -> p h c", c=64))
                nc.vector.memset(dst[:, :, 64:65], 1.0)

            for g in range(4):
                emit_qk_ct(g)
                emit_qk_ct(g + 4)
                for lt in range(4 * g, 4 * g + 4):
                    emit_v_lt(lt)

        # ---- phase 2+3: attention, out-proj folded in per q-chunk -----------
        with tc.tile_pool(name="att", bufs=2) as att, \
             tc.tile_pool(name="ph3", bufs=3) as ph3, \
             tc.tile_pool(name="sps", bufs=1, space="PSUM") as sps, \
             tc.tile_pool(name="avps", bufs=1, space="PSUM") as avps, \
             tc.tile_pool(name="bcps", bufs=1, space="PSUM") as bcps, \
             tc.tile_pool(name="p3ps", bufs=1, space="PSUM") as p3ps:
            for c in range(NQC):
                for p in range(4):       # head pairs (2p, 2p+1)
                    local = p < 2
                    nk = _nk(2 * p, c)
                    qsl = slice(c * QC, (c + 1) * QC)
                    eA = att.tile([128, NKT * QC], DT, tag="eA", name="eA")
                    eB = att.tile([128, NKT * QC], DT, tag="eB", name="eB")
                    # two k-tiles per PSUM tile (2 banks) -> one exp per pair
                    for kt2 in range(0, nk, 2):
                        psA = sps.tile([128, 2 * QC], DTF, tag="sA", name="sA")
                        psB = sps.tile([128, 2 * QC], DTF, tag="sB", name="sB")
                        for u in range(2):
                            kt = kt2 + u
                            ksl = slice(kt * 128, (kt + 1) * 128)
                            usl = slice(u * QC, (u + 1) * QC)
                            nc.tensor.matmul(psA[:, usl], qk[4 + p][0:64, ksl],
                                             qk[p][0:64, qsl],
                                             start=True, stop=True,
                                             tile_position=(0, 0))
                            nc.tensor.matmul(psB[:, usl], qk[4 + p][64:128, ksl],
                                             qk[p][64:128, qsl],
                                             start=True, stop=True,
                                             tile_position=(64, 0))
                        esl = slice(kt2 * QC, (kt2 + 2) * QC)
                        nc.scalar.activation(eA[:, esl], psA[:], AF.Exp)
                        nc.scalar.activation(eB[:, esl], psB[:], AF.Exp)
                    if local:  # mask diagonal band (last 4 computed k-tiles)
                        dsl = slice((nk - 4) * QC, nk * QC)
                        nc.vector.tensor_tensor(eA[:, dsl], eA[:, dsl],
                                                masks[:], ALU.mult)
                        nc.vector.tensor_tensor(eB[:, dsl], eB[:, dsl],
                                                masks[:], ALU.mult)
                    pvA = avps.tile([65, QC], DTF, tag="avA", name="avA")
                    pvB = avps.tile([65, QC], DTF, tag="avB", name="avB")
                    for kt in range(nk):
                        esl = slice(kt * QC, (kt + 1) * QC)
                        hA, hB = 2 * p, 2 * p + 1
                        nc.tensor.matmul(pvA[:], v_aug[kt][:, hA * 65:(hA + 1) * 65],
                                         eA[:, esl], start=(kt == 0),
                                         stop=(kt == nk - 1))
                        nc.tensor.matmul(pvB[:], v_aug[kt][:, hB * 65:(hB + 1) * 65],
                                         eB[:, esl], start=(kt == 0),
                                         stop=(kt == nk - 1))
                    for hh, e_t, pv in ((0, eA, pvA), (1, eB, pvB)):
                        # 1/denom = exp(-ln(denom)) on ACT (both funcs live in
                        # the natural_log_exp_and_others table set), then
                        # broadcast across partitions with a K=1 ones-matmul.
                        ln_d = att.tile([1, QC], DTF, tag=f"ln_d{hh}", name=f"ln_d{hh}")
                        rec_b = att.tile([1, QC], DT, tag=f"rec_b{hh}", name=f"rec_b{hh}")
                        rb_b = att.tile([128, QC], DT, tag=f"rb_b{hh}", name=f"rb_b{hh}")
                        nc.scalar.activation(ln_d[:], pv[64:65, :], AF.Ln)
                        nc.scalar.activation(rec_b[:], ln_d[:], AF.Exp, scale=-1.0)
                        psb = bcps.tile([128, QC], DTF, tag="psb", name="psb")
                        nc.tensor.matmul(psb[:], ones[0:1, 0:128], rec_b[:],
                                         start=True, stop=True)
                        nc.vector.tensor_copy(rb_b[:], psb[:])
                        # normalize O^T slice into ocT (rb_b holds the exact
                        # bf16 rec values; the f32 copy would be identical)
                        osl = slice(hh * 64, (hh + 1) * 64)
                        nc.vector.tensor_tensor(oc[p][osl, qsl], pv[0:64, :],
                                                rb_b[0:64, :], ALU.mult)
                        # normalize E^T in place (bf16, 2x mode)
                        e3 = e_t[:, 0:nk * QC].rearrange("p (n f) -> p n f", f=QC)
                        r3 = rb_b[:].rearrange("p (n f) -> p n f", f=QC) \
                                    .to_broadcast((128, nk, QC))
                        nc.vector.tensor_tensor(e3, e3, r3, ALU.mult)
                        # attn block out
                        ch = 2 * p + hh
                        off = _BLOCK_OFFS[(ch, c)]
                        dram = attn_d[off:off + 128 * nk * QC] \
                            .rearrange("(p f) -> p f", f=nk * QC)
                        nc.sync.dma_start(dram, e_t[:, 0:nk * QC])

                # out-proj for this q-chunk (all pairs done for chunk c)
                for lt in range(4 * c, 4 * c + 4):
                    lsl = slice(lt * 128, (lt + 1) * 128)
                    ob = ph3.tile([128, 1024], DTF, tag="ob", name="ob")
                    for ncol in range(2):
                        nsl = slice(ncol * 512, (ncol + 1) * 512)
                        ps = p3ps.tile([128, 512], DTF, tag="ps3", name="ps3")
                        for t in range(4):
                            nc.tensor.matmul(ps[:], oc[t][:, lsl], w2[t][:, nsl],
                                             start=(t == 0), stop=(t == 3))
                        nc.vector.tensor_copy(ob[:, nsl], ps[:])
                    nc.sync.dma_start(outp_d[lsl, :], ob[:])


# ---------------------------------------------------------------------------
# Host side
# ---------------------------------------------------------------------------
_CACHE = {}


def _get_nc(has_bias=False):
    key = ("nc", has_bias)
    if key not in _CACHE:
        _CACHE[key] = _build_bass(has_bias)
    return _CACHE[key]


def _core_heads(j):
    """Global head indices (0..15) handled by head-half j, in core order."""
    return [4 * j + k for k in range(4)] + [8 + 4 * j + k for k in range(4)]


def _make_masks():
    m = np.zeros((128, 2048), np.float32)
    for jj in range(4):
        kp = np.arange(128)[:, None]
        qf = np.arange(512)[None, :]
        m[:, jj * 512:(jj + 1) * 512] = (qf >= 128 * jj + kp)
    return m.astype(BF16)


def kernel(x, qkv_w, qkv_b, out_w, out_b, local_scale, global_scale):
    x = np.asarray(x, np.float32)
    qkv_w = np.asarray(qkv_w, np.float32)
    qkv_b = np.asarray(qkv_b, np.float32)
    out_w = np.asarray(out_w, np.float32)
    out_b = np.asarray(out_b, np.float32)
    ls = float(np.asarray(local_scale).reshape(-1)[0])
    gs = float(np.asarray(global_scale).reshape(-1)[0])

    masks = _make_masks()
    w3 = qkv_w.reshape(D, 3, H, HD)
    b3 = qkv_b.reshape(3, H, HD)

    in_maps = []
    for i in range(8):
        b, j = divmod(i, 2)
        heads = _core_heads(j)
        w_q = np.concatenate([w3[:, 0, h, :] for h in heads], axis=1)
        w_k = np.concatenate([w3[:, 1, h, :] for h in heads], axis=1)
        w_v = np.concatenate([w3[:, 2, h, :] for h in heads], axis=1)
        b_q = np.concatenate([b3[0, h, :] for h in heads])
        b_k = np.concatenate([b3[1, h, :] for h in heads])
        b_v = np.concatenate([b3[2, h, :] for h in heads])
        w2 = np.concatenate([out_w[h * HD:(h + 1) * HD, :] for h in heads], axis=0)
        scl = np.empty((512, 1), np.float32)
        scl[:256] = ls / math.sqrt(HD)
        scl[256:] = gs / math.sqrt(HD)
        in_maps.append({
            "xT": np.ascontiguousarray(x[b].T).astype(BF16),
            "w_qk": np.concatenate([w_q, w_k], axis=1).astype(BF16),
            "w_v": w_v.astype(BF16),
            "b_qk": np.concatenate([b_q, b_k])[None, :].astype(BF16),
            "b_v": b_v[None, :].astype(BF16),
            "scl_q": scl,
            "w2": w2.astype(BF16),
            "masks": masks,
        })

    has_bias = bool(np.any(qkv_b))
    nc = _get_nc(has_bias)
    res = run_bass_kernel_spmd(nc, in_maps, list(range(8))).results

    out = np.zeros((B, L, D), np.float32)
    attn_local = np.zeros((B, LH, L, L), np.float32)
    attn_global = np.zeros((B, LH, L, L), np.float32)
    for i in range(8):
        b, j = divmod(i, 2)
        out[b] += np.asarray(res[i]["out_p"], np.float32)
        at = np.asarray(res[i]["attn_t"])
        for ch in range(CH):
            for c in range(NQC):
                nk = _nk(ch, c)
                off = _BLOCK_OFFS[(ch, c)]
                blk = at[off:off + 128 * nk * QC].reshape(128, nk, QC)
                sub = blk.transpose(2, 1, 0).reshape(QC, nk * 128)
                sub = sub.astype(np.float32)
                if ch < 4:
                    attn_local[b, 4 * j + ch, c * QC:(c + 1) * QC, :nk * 128] = sub
                else:
                    attn_global[b, 4 * j + ch - 4, c * QC:(c + 1) * QC, :nk * 128] = sub
    out += out_b[None, None, :]
    return out, attn_local, attn_global


# revision 35
# speedup vs baseline: 1.0177x; 1.0177x over previous
"""Bicameral attention (8 local-causal + 8 global heads) on 8 TRN2 NeuronCores.

Sharding: core i handles batch b = i//2 and head-half j = i%2, i.e. 4 local
heads {4j..4j+3} and 4 global heads {8+4j..8+4j+3} — identical SPMD program,
per-core data prepared on the host.

Per-core device pipeline (all matmuls bf16, fp32 PSUM accumulation):
  1. qkT [1024, L]  = w_qk.T @ xT   (Q^T and K^T, head-dim on partitions;
     biases folded in with K=1 rank-1 matmuls; softmax scale folded into Q^T)
     v   [L, 520]   = xT.T @ w_v    (65-wide per head: 64 V columns + ones
     column so attn@V also produces softmax denominators)
  2. per (head-pair, 512-wide q-chunk):
     S^T tiles [k,q] via row-packed K=64 matmul pairs -> exp on ACT -> bf16
     E^T in SBUF -> causal mask multiply (local heads, diagonal band only)
     -> attn@V with M=65 (PSUM row 64 = denominators) -> reciprocal ->
     gpsimd partition_broadcast -> DVE normalizes E^T in place and the
     O^T slice into ocT -> one contiguous DMA of [128, nk*512] to DRAM.
     Fully-masked k-tiles are never computed or written (host leaves zeros).
  3. out_p [L, 1024] = ocT.T @ w2 — partial output projection; host sums the
     two cores of each batch and adds out_b.

Host reassembles attn blocks (transposing [k,q] -> [q,k]) and upcasts to f32.
"""

import math
import numpy as np
import ml_dtypes

import concourse.bass as bass
import concourse.bacc as bacc
import concourse.mybir as mybir
from concourse import tile
from concourse.bass_utils import run_bass_kernel_spmd
from concourse.vector_clock import ScopedClock

BF16 = ml_dtypes.bfloat16
F32 = np.float32

B, L, D = 4, 2048, 1024
H, HD = 16, 64
LH = H // 2              # 8 local heads / 8 global heads
CH = 8                   # heads per core (4 local + 4 global)
NQC = 4                  # q-chunks of 512
QC = 512
KT = 128                 # k-tile size
NKT = L // KT            # 16 k-tiles
DT = mybir.dt.bfloat16
DTF = mybir.dt.float32
AF = mybir.ActivationFunctionType
ALU = mybir.AluOpType


def _nk(ch, c):
    """Number of computed k-tiles for core-head ch, q-chunk c."""
    return 4 * (c + 1) if ch < 4 else NKT


# DRAM layout of the attn output: per (ch, c) a contiguous [128, nk*512]
# block (partition-major), concatenated.
_BLOCK_OFFS = {}
_TOTAL_ATTN = 0


def _init_offs():
    global _TOTAL_ATTN
    off = 0
    for ch in range(CH):
        for c in range(NQC):
            _BLOCK_OFFS[(ch, c)] = off
            off += 128 * _nk(ch, c) * QC
    _TOTAL_ATTN = off


_init_offs()


# ---------------------------------------------------------------------------
# Workaround: walrus CoreV3 codegen accepts at most one sync-wait on a Drain.
# Move the TileContext tail-drain waits onto individual wait_ge instructions.
# ---------------------------------------------------------------------------
def _patched_drain_and_barrier(self, tick_clock, wait_clock):
    nc = self.nc
    drain_inst = nc.sync.drain()
    wait_clock.add_sem_waits(
        drain_inst.ins, ScopedClock({None: tick_clock.global_clock})
    )
    waits = list(drain_inst.ins.sync_info.on_wait)
    if len(waits) > 1:
        drain_inst.ins.sync_info.on_wait.clear()
        num_to_handle = {h.num: h for h in self.sems.allocated().values()}
        for w in waits:
            handle = num_to_handle.get(w.id)
            assert handle is not None, f"unknown tail sem {w.ant_name}"
            assert w.wait_mode == "sem-ge-imm", w.wait_mode
            nc.sync.wait_ge(handle, w.wait_value)
    nc.all_engine_barrier()
    popped = nc._tile_sem_poison_stack.pop()
    assert popped is self._sem_poison
    nc.clear_and_free_semaphores(list(self.sems.allocated().values()))
    nc.all_engine_barrier()


tile.TileContext._drain_and_barrier = _patched_drain_and_barrier


# The kernel alternates Exp (softmax) and Ln (reciprocal via exp(-ln(d))).
# Left alone, insert_act_table_loads maps Exp to `exp_and_others` and Ln to a
# natural-log set, reloading the ACT spline tables (~1.3us each) dozens of
# times. `natural_log_exp_and_others` holds both; restrict Exp/Ln to it (set
# ids are positional, so contents are filtered rather than reordered).
from concourse.hw_specs import get_activation_tables as _orig_act_tables


def _patched_act_tables(arch):
    keep = "natural_log_exp_and_others"
    out = {}
    for name, fns in _orig_act_tables(arch).items():
        if name != keep:
            fns = {f for f in fns if f not in (AF.Exp, AF.Ln)}
        out[name] = set(fns)
    return out


bacc.get_activation_tables = _patched_act_tables


# ---------------------------------------------------------------------------
# Device program (identical on every core)
# ---------------------------------------------------------------------------
def _build_bass(has_bias):
    nc = bacc.Bacc("TRN2", target_bir_lowering=False, debug=False, num_devices=8)

    xT_d = nc.dram_tensor("xT", [D, L], DT, kind="ExternalInput").ap()
    wqk_d = nc.dram_tensor("w_qk", [D, 1024], DT, kind="ExternalInput").ap()
    wv_d = nc.dram_tensor("w_v", [D, 512], DT, kind="ExternalInput").ap()
    bqk_d = nc.dram_tensor("b_qk", [1, 1024], DT, kind="ExternalInput").ap()
    bv_d = nc.dram_tensor("b_v", [1, 512], DT, kind="ExternalInput").ap()
    scl_d = nc.dram_tensor("scl_q", [512, 1], DTF, kind="ExternalInput").ap()
    w2_d = nc.dram_tensor("w2", [512, 1024], DT, kind="ExternalInput").ap()
    masks_d = nc.dram_tensor("masks", [128, 2048], DT, kind="ExternalInput").ap()

    attn_d = nc.dram_tensor("attn_t", [_TOTAL_ATTN], DT, kind="ExternalOutput").ap()
    outp_d = nc.dram_tensor("out_p", [L, D], DTF, kind="ExternalOutput").ap()

    with tile.TileContext(nc) as tc:
        _emit(nc, tc, xT_d, wqk_d, wv_d, bqk_d, bv_d, scl_d, w2_d, masks_d,
              attn_d, outp_d, has_bias)
    nc.compile()
    return nc


def _emit(nc, tc, xT_d, wqk_d, wv_d, bqk_d, bv_d, scl_d, w2_d, masks_d,
          attn_d, outp_d, has_bias):
    from contextlib import ExitStack

    ctx = ExitStack()
    with ctx:
        persist = ctx.enter_context(tc.tile_pool(name="persist", bufs=1))

        # ---- persistent tiles ------------------------------------------------
        qk = [persist.tile([128, L], DT, tag=f"qk{t}", name=f"qk{t}") for t in range(8)]
        v_aug = [persist.tile([128, 520], DT, tag=f"v{t}", name=f"v{t}") for t in range(NKT)]
        oc = [persist.tile([128, L], DT, tag=f"oc{t}", name=f"oc{t}") for t in range(4)]
        w2 = [persist.tile([128, 1024], DT, tag=f"w2{t}", name=f"w2{t}") for t in range(4)]
        masks = persist.tile([128, 2048], DT, tag="masks", name="masks")
        scl = [persist.tile([128, 1], DTF, tag=f"scl{t}", name=f"scl{t}") for t in range(4)]
        ones = persist.tile([1, 512], DT, tag="ones", name="ones")
        bqk = persist.tile([1, 1024], DT, tag="bqk", name="bqk")
        bv = persist.tile([1, 512], DT, tag="bv", name="bv")

        nc.vector.memset(ones[:], 1.0)

        # ---- phase 1: qkv projection ----------------------------------------
        with tc.tile_pool(name="ph1", bufs=1) as ph1, \
             tc.tile_pool(name="p1ps", bufs=4, space="PSUM") as p1ps:
            xt = [ph1.tile([128, L], DT, tag=f"xt{d}", name=f"xt{d}") for d in range(8)]
            wqk = [ph1.tile([128, 1024], DT, tag=f"wqk{d}", name=f"wqk{d}") for d in range(8)]
            wv = [ph1.tile([128, 512], DT, tag=f"wv{d}", name=f"wv{d}") for d in range(8)]
            # first-needed tiles first: the very first matmul touches only
            # xt[0] + wqk[0]; bias/scale rows follow shortly after.
            nc.sync.dma_start(xt[0][:], xT_d[0:128, :])
            nc.sync.dma_start(wqk[0][:], wqk_d[0:128, :])
            if has_bias:
                nc.sync.dma_start(bqk[:], bqk_d[:])
            for t in range(4):
                nc.sync.dma_start(scl[t][:], scl_d[t * 128:(t + 1) * 128, :])
            for d in range(1, 8):
                nc.sync.dma_start(xt[d][:], xT_d[d * 128:(d + 1) * 128, :])
                nc.sync.dma_start(wqk[d][:], wqk_d[d * 128:(d + 1) * 128, :])
            for d in range(8):
                nc.sync.dma_start(wv[d][:], wv_d[d * 128:(d + 1) * 128, :])
            if has_bias:
                nc.sync.dma_start(bv[:], bv_d[:])
            nc.sync.dma_start(masks[:], masks_d[:])
            for t in range(4):
                nc.sync.dma_start(w2[t][:], w2_d[t * 128:(t + 1) * 128, :])

            # qkT[c, l] += w_qk[:, c].T @ xT ; optional bias via K=1 outer
            # product. Q/K c-tiles interleaved so head-pair 0's operands
            # finish first, with V L-tiles woven in so early attn@V chunks
            # (local heads need only the first few v tiles) aren't blocked
            # behind the whole projection.
            def emit_qk_ct(ct):
                csl = slice(ct * 128, (ct + 1) * 128)
                for lc in range(NQC):
                    lsl = slice(lc * QC, (lc + 1) * QC)
                    ps = p1ps.tile([128, QC], DTF, tag="ps", name="ps")
                    for d in range(8):
                        nc.tensor.matmul(ps[:], wqk[d][:, csl], xt[d][:, lsl],
                                         start=(d == 0), stop=not has_bias and d == 7)
                    if has_bias:
                        nc.tensor.matmul(ps[:], bqk[0:1, csl], ones[0:1, 0:QC],
                                         start=False, stop=True)
                    if ct < 4:  # Q^T rows: fold softmax scale
                        nc.vector.tensor_scalar(qk[ct][:, lsl], ps[:],
                                                scl[ct][:], None, ALU.mult)
                    else:
                        nc.vector.tensor_copy(qk[ct][:, lsl], ps[:])

            def emit_v_lt(lt):
                # v[l, 520] (65 per head: 64 cols + ones col)
                lsl = slice(lt * 128, (lt + 1) * 128)
                ps = p1ps.tile([128, 512], DTF, tag="psv", name="psv")
                for d in range(8):
                    nc.tensor.matmul(ps[:], xt[d][:, lsl], wv[d][:],
                                     start=(d == 0), stop=not has_bias and d == 7)
                if has_bias:
                    nc.tensor.matmul(ps[:], ones[0:1, 0:128], bv[0:1, :],
                                     start=False, stop=True)
                dst = v_aug[lt][:].rearrange("p (h c) -> p h c", c=65)
                nc.vector.tensor_copy(dst[:, :, 0:64],
                                      ps[:].rearrange("p (h c) -> p h c", c=64))
                nc.vector.memset(dst[:, :, 64:65], 1.0)

            for g in range(4):
                emit_qk_ct(g)
                emit_qk_ct(g + 4)
                for lt in range(4 * g, 4 * g + 4):
                    emit_v_lt(lt)

        # ---- phase 2+3: attention, out-proj folded in per q-chunk -----------
        with tc.tile_pool(name="att", bufs=2) as att, \
             tc.tile_pool(name="ph3", bufs=2) as ph3, \
             tc.tile_pool(name="sps", bufs=1, space="PSUM") as sps, \
             tc.tile_pool(name="avps", bufs=1, space="PSUM") as avps, \
             tc.tile_pool(name="bcps", bufs=1, space="PSUM") as bcps, \
             tc.tile_pool(name="p3ps", bufs=1, space="PSUM") as p3ps:
            for c in range(NQC):
                for p in range(4):       # head pairs (2p, 2p+1)
                    local = p < 2
                    nk = _nk(2 * p, c)
                    qsl = slice(c * QC, (c + 1) * QC)
                    eA = att.tile([128, NKT * QC], DT, tag="eA", name="eA", bufs=3)
                    eB = att.tile([128, NKT * QC], DT, tag="eB", name="eB", bufs=3)
                    # two k-tiles per PSUM tile (2 banks) -> one exp per pair
                    for kt2 in range(0, nk, 2):
                        psA = sps.tile([128, 2 * QC], DTF, tag="sA", name="sA")
                        psB = sps.tile([128, 2 * QC], DTF, tag="sB", name="sB")
                        for u in range(2):
                            kt = kt2 + u
                            ksl = slice(kt * 128, (kt + 1) * 128)
                            usl = slice(u * QC, (u + 1) * QC)
                            nc.tensor.matmul(psA[:, usl], qk[4 + p][0:64, ksl],
                                             qk[p][0:64, qsl],
                                             start=True, stop=True,
                                             tile_position=(0, 0))
                            nc.tensor.matmul(psB[:, usl], qk[4 + p][64:128, ksl],
                                             qk[p][64:128, qsl],
                                             start=True, stop=True,
                                             tile_position=(64, 0))
                        esl = slice(kt2 * QC, (kt2 + 2) * QC)
                        nc.scalar.activation(eA[:, esl], psA[:], AF.Exp)
                        nc.scalar.activation(eB[:, esl], psB[:], AF.Exp)
                    if local:  # mask diagonal band (last 4 computed k-tiles)
                        dsl = slice((nk - 4) * QC, nk * QC)
                        nc.vector.tensor_tensor(eA[:, dsl], eA[:, dsl],
                                                masks[:], ALU.mult)
                        nc.vector.tensor_tensor(eB[:, dsl], eB[:, dsl],
                                                masks[:], ALU.mult)
                    pvA = avps.tile([65, QC], DTF, tag="avA", name="avA")
                    pvB = avps.tile([65, QC], DTF, tag="avB", name="avB")
                    for kt in range(nk):
                        esl = slice(kt * QC, (kt + 1) * QC)
                        hA, hB = 2 * p, 2 * p + 1
                        nc.tensor.matmul(pvA[:], v_aug[kt][:, hA * 65:(hA + 1) * 65],
                                         eA[:, esl], start=(kt == 0),
                                         stop=(kt == nk - 1))
                        nc.tensor.matmul(pvB[:], v_aug[kt][:, hB * 65:(hB + 1) * 65],
                                         eB[:, esl], start=(kt == 0),
                                         stop=(kt == nk - 1))
                    for hh, e_t, pv in ((0, eA, pvA), (1, eB, pvB)):
                        # 1/denom = exp(-ln(denom)) on ACT (both funcs live in
                        # the natural_log_exp_and_others table set), then
                        # broadcast across partitions with a K=1 ones-matmul.
                        ln_d = att.tile([1, QC], DTF, tag=f"ln_d{hh}", name=f"ln_d{hh}")
                        rec_b = att.tile([1, QC], DT, tag=f"rec_b{hh}", name=f"rec_b{hh}")
                        rb_b = att.tile([128, QC], DT, tag=f"rb_b{hh}", name=f"rb_b{hh}")
                        nc.scalar.activation(ln_d[:], pv[64:65, :], AF.Ln)
                        nc.scalar.activation(rec_b[:], ln_d[:], AF.Exp, scale=-1.0)
                        psb = bcps.tile([128, QC], DTF, tag="psb", name="psb")
                        nc.tensor.matmul(psb[:], ones[0:1, 0:128], rec_b[:],
                                         start=True, stop=True)
                        nc.vector.tensor_copy(rb_b[:], psb[:])
                        # normalize O^T slice into ocT (rb_b holds the exact
                        # bf16 rec values; the f32 copy would be identical)
                        osl = slice(hh * 64, (hh + 1) * 64)
                        nc.vector.tensor_tensor(oc[p][osl, qsl], pv[0:64, :],
                                                rb_b[0:64, :], ALU.mult)
                        # normalize E^T in place (bf16, 2x mode)
                        e3 = e_t[:, 0:nk * QC].rearrange("p (n f) -> p n f", f=QC)
                        r3 = rb_b[:].rearrange("p (n f) -> p n f", f=QC) \
                                    .to_broadcast((128, nk, QC))
                        nc.vector.tensor_tensor(e3, e3, r3, ALU.mult)
                        # attn block out
                        ch = 2 * p + hh
                        off = _BLOCK_OFFS[(ch, c)]
                        dram = attn_d[off:off + 128 * nk * QC] \
                            .rearrange("(p f) -> p f", f=nk * QC)
                        nc.sync.dma_start(dram, e_t[:, 0:nk * QC])

                # out-proj for this q-chunk (all pairs done for chunk c)
                for lt in range(4 * c, 4 * c + 4):
                    lsl = slice(lt * 128, (lt + 1) * 128)
                    ob = ph3.tile([128, 1024], DTF, tag="ob", name="ob")
                    for ncol in range(2):
                        nsl = slice(ncol * 512, (ncol + 1) * 512)
                        ps = p3ps.tile([128, 512], DTF, tag="ps3", name="ps3")
                        for t in range(4):
                            nc.tensor.matmul(ps[:], oc[t][:, lsl], w2[t][:, nsl],
                                             start=(t == 0), stop=(t == 3))
                        nc.vector.tensor_copy(ob[:, nsl], ps[:])
                    nc.sync.dma_start(outp_d[lsl, :], ob[:])


# ---------------------------------------------------------------------------
# Host side
# ---------------------------------------------------------------------------
_CACHE = {}


def _get_nc(has_bias=False):
    key = ("nc", has_bias)
    if key not in _CACHE:
        _CACHE[key] = _build_bass(has_bias)
    return _CACHE[key]


def _core_heads(j):
    """Global head indices (0..15) handled by head-half j, in core order."""
    return [4 * j + k for k in range(4)] + [8 + 4 * j + k for k in range(4)]


def _make_masks():
    m = np.zeros((128, 2048), np.float32)
    for jj in range(4):
        kp = np.arange(128)[:, None]
        qf = np.arange(512)[None, :]
        m[:, jj * 512:(jj + 1) * 512] = (qf >= 128 * jj + kp)
    return m.astype(BF16)


def kernel(x, qkv_w, qkv_b, out_w, out_b, local_scale, global_scale):
    x = np.asarray(x, np.float32)
    qkv_w = np.asarray(qkv_w, np.float32)
    qkv_b = np.asarray(qkv_b, np.float32)
    out_w = np.asarray(out_w, np.float32)
    out_b = np.asarray(out_b, np.float32)
    ls = float(np.asarray(local_scale).reshape(-1)[0])
    gs = float(np.asarray(global_scale).reshape(-1)[0])

    masks = _make_masks()
    w3 = qkv_w.reshape(D, 3, H, HD)
    b3 = qkv_b.reshape(3, H, HD)

    in_maps = []
    for i in range(8):
        b, j = divmod(i, 2)
        heads = _core_heads(j)
        w_q = np.concatenate([w3[:, 0, h, :] for h in heads], axis=1)
        w_k = np.concatenate([w3[:, 1, h, :] for h in heads], axis=1)
        w_v = np.concatenate([w3[:, 2, h, :] for h in heads], axis=1)
        b_q = np.concatenate([b3[0, h, :] for h in heads])
        b_k = np.concatenate([b3[1, h, :] for h in heads])
        b_v = np.concatenate([b3[2, h, :] for h in heads])
        w2 = np.concatenate([out_w[h * HD:(h + 1) * HD, :] for h in heads], axis=0)
        scl = np.empty((512, 1), np.float32)
        scl[:256] = ls / math.sqrt(HD)
        scl[256:] = gs / math.sqrt(HD)
        in_maps.append({
            "xT": np.ascontiguousarray(x[b].T).astype(BF16),
            "w_qk": np.concatenate([w_q, w_k], axis=1).astype(BF16),
            "w_v": w_v.astype(BF16),
            "b_qk": np.concatenate([b_q, b_k])[None, :].astype(BF16),
            "b_v": b_v[None, :].astype(BF16),
            "scl_q": scl,
            "w2": w2.astype(BF16),
            "masks": masks,
        })

    has_bias = bool(np.any(qkv_b))
    nc = _get_nc(has_bias)
    res = run_bass_kernel_spmd(nc, in_maps, list(range(8))).results

    out = np.zeros((B, L, D), np.float32)
    attn_local = np.zeros((B, LH, L, L), np.float32)
    attn_global = np.zeros((B, LH, L, L), np.float32)
    for i in range(8):
        b, j = divmod(i, 2)
        out[b] += np.asarray(res[i]["out_p"], np.float32)
        at = np.asarray(res[i]["attn_t"])
        for ch in range(CH):
            for c in range(NQC):
                nk = _nk(ch, c)
                off = _BLOCK_OFFS[(ch, c)]
                blk = at[off:off + 128 * nk * QC].reshape(128, nk, QC)
                sub = blk.transpose(2, 1, 0).reshape(QC, nk * 128)
                sub = sub.astype(np.float32)
                if ch < 4:
                    attn_local[b, 4 * j + ch, c * QC:(c + 1) * QC, :nk * 128] = sub
                else:
                    attn_global[b, 4 * j + ch - 4, c * QC:(c + 1) * QC, :nk * 128] = sub
    out += out_b[None, None, :]
    return out, attn_local, attn_global
